# revision 53
# baseline (speedup 1.0000x reference)
"""Grouped-GEMM MoE kernel for Trainium2, expert-parallel across 8 NeuronCores.

Strategy (matches the module's expert-parallel path):
  - Host routes: tokens whose two top-k slots pick the SAME expert are
    deduplicated into one routed row with combined weight (saves ~12% of
    rows); rows are sorted by expert; core e gets expert e's rows padded
    to a common capacity C (multiple of 128).
  - Device (per core): Y = gelu(X @ W1 + b1) @ W2 + b2, scaled per-row by
    the routing weight. Two chained GEMMs on the PE array in bf16 with
    fp32 PSUM accumulation; gelu fused into the PSUM->SBUF eviction on the
    ACT engine. Weight DMAs are chunked so the first matmul can start
    after ~0.5 MB instead of ~8 MB.
  - Host combines: scatter rows back by token and sum the two slots.

Problem shapes (hardcoded per contract): B=4, S=4096, H=1024, F=2048, E=8,
TOPK=2.
"""

import sys

for _p in ("/opt/trn_rl_repo", "/opt/pypackages"):
    if _p not in sys.path:
        sys.path.insert(0, _p)

import ml_dtypes
import numpy as np

import concourse.bass as bass  # noqa: F401  (engine types come via bacc)
import concourse.mybir as mybir
import concourse.tile as tile
from concourse import bacc
from concourse.bass_utils import run_bass_kernel_spmd

H = 1024
F = 2048
E = 8
TOPK = 2
N_CORES = 8
P = 128
NTILE = 512          # full token-tile width (matmul moving dim)
KK1 = H // P         # 8  k-steps in GEMM1
KK2 = F // P         # 16 k-steps in GEMM2
MT1 = F // P         # 16 output m-tiles in GEMM1
NT2 = H // NTILE     # 2  output n-tiles in GEMM2

BF16 = mybir.dt.bfloat16
F32 = mybir.dt.float32

_CACHE = {}
last_result = None   # BassKernelResults of the most recent device run


def _tiles(C):
    """Token-tile (offset, width) list: full 512s plus one 128k remainder."""
    out = []
    off = 0
    while C - off >= NTILE:
        out.append((off, NTILE))
        off += NTILE
    if C - off:
        out.append((off, C - off))
    return out


def _build(C):
    """Build + compile the per-core program for capacity C (multiple of 128)."""
    assert C % P == 0
    nmc = C // P          # token m-tiles (for GEMM2 output rows)
    tiles = _tiles(C)

    nc = bacc.Bacc("TRN2", target_bir_lowering=False, debug=False,
                   num_devices=N_CORES)

    # DRAM I/O. Layouts are chosen so every DMA is >=2KB-contiguous per
    # partition line:
    #   xt[p, .]       = per-tile blocks [KK1, w] of X[c, kk*128+p] transposed
    #   w1[m, p, .]    = W1[kk*128+p, m*128+j] as [KK1, 128] per (m, p)
    #   w2[p, .]       = W2[kk*128+p, j] as [KK2, H]
    #   b1[p, m]       = b1[m*128+p]
    #   b2r[p, j]      = b2[j]            (pre-replicated across partitions)
    #   rw[p, mc]      = rweight[mc*128+p]
    #   y[mc, p, j]    = Y[mc*128+p, j]
    xt_d = nc.dram_tensor("xt", [P, KK1 * C], BF16, kind="ExternalInput")
    w1_d = nc.dram_tensor("w1", [MT1, P, KK1 * P], BF16, kind="ExternalInput")
    w2_d = nc.dram_tensor("w2", [P, KK2 * H], BF16, kind="ExternalInput")
    b1_d = nc.dram_tensor("b1", [P, MT1], F32, kind="ExternalInput")
    rw_d = nc.dram_tensor("rw", [P, nmc], F32, kind="ExternalInput")
    y_d = nc.dram_tensor("y", [nmc, P, H], BF16, kind="ExternalOutput")

    with tile.TileContext(nc) as tc:
        with (
            tc.tile_pool(name="const", bufs=1) as const,
            tc.tile_pool(name="xin", bufs=3) as xin,
            tc.tile_pool(name="gact", bufs=4) as gact,
            tc.tile_pool(name="yout", bufs=4) as yout,
            tc.tile_pool(name="psg", bufs=3, space="PSUM") as psg,
            tc.tile_pool(name="psy", bufs=5, space="PSUM") as psy,
        ):
            b1_sb = const.tile([P, MT1], F32)
            rw_sb = const.tile([P, nmc], F32)
            # One tile per weight chunk: Tile deps are per-tile, so a single
            # big w1 tile would make the first matmul wait for ALL 16 DMAs.
            w1m = [const.tile([P, KK1 * P], BF16, name=f"w1m{m}")
                   for m in range(MT1)]
            w2k = [const.tile([P, H], BF16, name=f"w2k{kk}")
                   for kk in range(KK2)]
            junk = const.tile([P, P], BF16)     # PE warm-up food
            junk2 = const.tile([P, NTILE], BF16)

            # Warm the PE's HAM clock gate while the first DMAs land, so the
            # real GEMM stream starts at 2.4 GHz with no idle window.
            nc.gpsimd.memset(junk[:], 0.0)
            nc.gpsimd.memset(junk2[:], 0.0)
            for i in range(20):
                pw = psy.tile([P, NTILE], F32, tag="py", name=f"pw{i}")
                nc.tensor.matmul(pw[:], junk[:], junk2[:],
                                 start=True, stop=True)

            xts = {}
            def xt_fetch(t, pieces=1, engs=None, separate=False):
                off, w = tiles[t]
                # pieces>1 fans the transfer across several DMA engines
                # (a single descriptor stream runs at ~145 GB/s);
                # separate=True makes each piece its own tile so matmuls
                # depend only on the kk-slices they actually read.
                if separate:
                    kkpp = KK1 // pieces
                    segs = []
                    for p_ in range(pieces):
                        seg = xin.tile([P, kkpp * w], BF16, tag=f"xtp{p_}",
                                       name=f"xt{t}_p{p_}")
                        nc.scalar.dma_start(
                            seg[:],
                            xt_d[:, KK1 * off + p_ * kkpp * w:
                                 KK1 * off + (p_ + 1) * kkpp * w])
                        segs.append(seg)
                    xts[t] = segs
                    return
                xts[t] = xin.tile([P, KK1 * w], BF16, tag="xt",
                                  name=f"xt{t}")
                span = KK1 * w
                step = span // pieces
                for p_ in range(pieces):
                    eng = (engs or [nc.scalar])[p_ % len(engs or [1])]
                    eng.dma_start(
                        xts[t][:, p_ * step:(p_ + 1) * step],
                        xt_d[:, KK1 * off + p_ * step:
                             KK1 * off + (p_ + 1) * step])

            # Two HWDGE queues: weights + even y-stores ride the Sync queue,
            # token tiles + odd y-stores ride the Activation queue.
            # Critical-path DMAs first: w1 m0/m1 + the first token tile,
            # then the weight refill stream (b1 after xt0 — it is not
            # needed until the first gelu eviction).
            nc.sync.dma_start(w1m[0][:], w1_d[0])
            nc.sync.dma_start(w1m[1][:], w1_d[1])
            xt_fetch(0, pieces=4)
            nc.scalar.dma_start(b1_sb[:], b1_d[:])
            for m in range(2, MT1):
                nc.sync.dma_start(w1m[m][:], w1_d[m])
            xt_fetch(1, pieces=2)
            nc.sync.dma_start(rw_sb[:], rw_d[:])
            for kk in range(KK2):
                nc.sync.dma_start(
                    w2k[kk][:], w2_d[:, kk * H:(kk + 1) * H])

            for t, (off, w) in enumerate(tiles):
                if t not in xts:
                    xt_fetch(t)
                if t + 2 < len(tiles) and t + 2 not in xts:
                    xt_fetch(t + 2)
                xt_sb = xts.pop(t)
                if isinstance(xt_sb, list):
                    kkpp = KK1 // len(xt_sb)
                    xseg = lambda kk: xt_sb[kk // kkpp][
                        :, (kk % kkpp) * w:(kk % kkpp + 1) * w]
                else:
                    xseg = lambda kk: xt_sb[:, kk * w:(kk + 1) * w]

                # GEMM1: GT[f, c] = sum_h W1[h, f] * XT[h, c], then
                # gelu(.+b1) on eviction. F on partitions, tokens on free.
                gt_sb = gact.tile([P, MT1 * w], BF16, tag="gt")
                for m in range(MT1):
                    pg = psg.tile([P, w], F32, tag="pg")
                    for kk in range(KK1):
                        nc.tensor.matmul(
                            pg[:],
                            w1m[m][:, kk * P:(kk + 1) * P],
                            xseg(kk),
                            start=(kk == 0), stop=(kk == KK1 - 1))
                    nc.scalar.activation(
                        gt_sb[:, m * w:(m + 1) * w], pg[:],
                        mybir.ActivationFunctionType.Gelu,
                        bias=b1_sb[:, m:m + 1])

                # GEMM2: Y[c, j] = sum_f GT[f, c] * W2[f, j]; tokens on
                # partitions. Evict with the routing-weight scale (b2 is
                # added host-side), then store the full H row-block with a
                # single DMA.
                for mo in range(w // P):
                    mc = off // P + mo
                    yo = yout.tile([P, H], BF16, tag="yo")
                    for n in range(NT2):
                        py = psy.tile([P, NTILE], F32, tag="py")
                        for kk in range(KK2):
                            nc.tensor.matmul(
                                py[:],
                                gt_sb[:, kk * w + mo * P:kk * w + (mo + 1) * P],
                                w2k[kk][:, n * NTILE:(n + 1) * NTILE],
                                start=(kk == 0), stop=(kk == KK2 - 1))
                        if mc >= nmc - 2:
                            # kernel tail: evict on the idle DVE so the ACT
                            # engine is free to enqueue the final stores
                            nc.vector.tensor_scalar_mul(
                                yo[:, n * NTILE:(n + 1) * NTILE], py[:],
                                rw_sb[:, mc:mc + 1])
                        else:
                            # evict on ACT: keeps the DVE out of the
                            # steady-state pipeline entirely
                            nc.scalar.activation(
                                yo[:, n * NTILE:(n + 1) * NTILE], py[:],
                                mybir.ActivationFunctionType.Copy,
                                scale=rw_sb[:, mc:mc + 1])
                    if mc >= nmc - 3:
                        # tail: split the store across both queues and four
                        # DMA engines so the final drain flushes fast
                        q = H // 4
                        for pi in range(4):
                            qe = nc.sync if pi % 2 == 0 else nc.scalar
                            qe.dma_start(y_d[mc, :, pi * q:(pi + 1) * q],
                                         yo[:, pi * q:(pi + 1) * q])
                    else:
                        qeng = nc.sync if mc % 2 == 0 else nc.scalar
                        qeng.dma_start(y_d[mc], yo[:])

    nc.compile()
    return nc


def kernel(hidden_states, expert_weights, top_experts, w1, b1, w2, b2,
           _trace=False):
    global last_result
    x = np.asarray(hidden_states, dtype=np.float32)
    fw = np.asarray(expert_weights, dtype=np.float32)
    te = np.asarray(top_experts).astype(np.int64)
    w1 = np.asarray(w1, dtype=np.float32)
    b1 = np.asarray(b1, dtype=np.float32)
    w2 = np.asarray(w2, dtype=np.float32)
    b2 = np.asarray(b2, dtype=np.float32)

    b, s, h = x.shape
    T = b * s
    xf = x.reshape(T, h)

    # Routed rows with same-expert dedup: row i<T is token i's slot-0 row
    # (combined weight when both slots agree); rows T.. are slot-1 rows of
    # the non-duplicate tokens.
    dup = te[:, 0] == te[:, 1]
    tok_nd = np.nonzero(~dup)[0]
    rows_tok = np.concatenate([np.arange(T), tok_nd])
    rows_e = np.concatenate([te[:, 0], te[tok_nd, 1]])
    rows_w = np.concatenate([np.where(dup, fw[:, 0] + fw[:, 1], fw[:, 0]),
                             fw[tok_nd, 1]])

    order = np.argsort(rows_e, kind="stable")
    counts = np.bincount(rows_e, minlength=E)
    starts = np.concatenate([[0], np.cumsum(counts)])
    C = max(int(-(-counts.max() // P)) * P, P)

    key = C
    if key not in _CACHE:
        _CACHE[key] = _build(C)
    nc = _CACHE[key]

    tiles = _tiles(C)
    nmc = C // P
    in_maps = []
    for e in range(E):
        idx = order[starts[e]:starts[e + 1]]
        cnt = len(idx)
        xe = np.zeros((C, H), np.float32)
        xe[:cnt] = xf[rows_tok[idx]]
        xt3 = np.ascontiguousarray(
            xe.T.reshape(KK1, P, C).transpose(1, 0, 2)).astype(
                ml_dtypes.bfloat16)                       # [P, KK1, C]
        xt = np.concatenate(
            [xt3[:, :, off:off + w].reshape(P, KK1 * w)
             for off, w in tiles], axis=1)                # [P, KK1*C]
        rwe = np.zeros(C, np.float32)
        rwe[:cnt] = rows_w[idx]
        w1e = np.ascontiguousarray(
            w1[e].reshape(KK1, P, MT1, P).transpose(2, 1, 0, 3).reshape(
                MT1, P, KK1 * P)).astype(ml_dtypes.bfloat16)
        in_maps.append({
            "xt": np.ascontiguousarray(xt),
            "w1": w1e,
            "w2": np.ascontiguousarray(
                w2[e].reshape(KK2, P, H).transpose(1, 0, 2).reshape(
                    P, KK2 * H)).astype(ml_dtypes.bfloat16),
            "b1": np.ascontiguousarray(b1[e].reshape(MT1, P).T),
            "rw": np.ascontiguousarray(rwe.reshape(nmc, P).T),
        })

    res = run_bass_kernel_spmd(nc, in_maps, list(range(N_CORES)),
                               trace=_trace)
    last_result = res

    nrows = len(rows_tok)
    routed = np.zeros((nrows, H), np.float32)
    for e in range(E):
        idx = order[starts[e]:starts[e + 1]]
        cnt = len(idx)
        ye = np.asarray(res.results[e]["y"]).astype(np.float32).reshape(
            C, H)[:cnt]
        # b2 is applied here (scaled by the routing weight) rather than on
        # the device, saving one vector op per eviction.
        routed[idx] = ye + np.outer(rows_w[idx], b2[e])

    y = routed[:T]
    y[tok_nd] += routed[T:]
    return y.reshape(b, s, h).astype(np.float32)


# revision 54
# speedup vs baseline: 1.0084x; 1.0084x over previous
"""Grouped-GEMM MoE kernel for Trainium2, expert-parallel across 8 NeuronCores.

Strategy (matches the module's expert-parallel path):
  - Host routes: tokens whose two top-k slots pick the SAME expert are
    deduplicated into one routed row with combined weight (saves ~12% of
    rows); rows are sorted by expert; core e gets expert e's rows padded
    to a common capacity C (multiple of 128).
  - Device (per core): Y = gelu(X @ W1 + b1) @ W2 + b2, scaled per-row by
    the routing weight. Two chained GEMMs on the PE array in bf16 with
    fp32 PSUM accumulation; gelu fused into the PSUM->SBUF eviction on the
    ACT engine. Weight DMAs are chunked so the first matmul can start
    after ~0.5 MB instead of ~8 MB.
  - Host combines: scatter rows back by token and sum the two slots.

Problem shapes (hardcoded per contract): B=4, S=4096, H=1024, F=2048, E=8,
TOPK=2.
"""

import sys

for _p in ("/opt/trn_rl_repo", "/opt/pypackages"):
    if _p not in sys.path:
        sys.path.insert(0, _p)

import ml_dtypes
import numpy as np

import concourse.bass as bass  # noqa: F401  (engine types come via bacc)
import concourse.mybir as mybir
import concourse.tile as tile
from concourse import bacc
from concourse.bass_utils import run_bass_kernel_spmd

H = 1024
F = 2048
E = 8
TOPK = 2
N_CORES = 8
P = 128
NTILE = 512          # full token-tile width (matmul moving dim)
KK1 = H // P         # 8  k-steps in GEMM1
KK2 = F // P         # 16 k-steps in GEMM2
MT1 = F // P         # 16 output m-tiles in GEMM1
NT2 = H // NTILE     # 2  output n-tiles in GEMM2

BF16 = mybir.dt.bfloat16
F32 = mybir.dt.float32

_CACHE = {}
last_result = None   # BassKernelResults of the most recent device run


def _tiles(C):
    """Token-tile (offset, width) list: full 512s plus one 128k remainder."""
    out = []
    off = 0
    while C - off >= NTILE:
        out.append((off, NTILE))
        off += NTILE
    if C - off:
        out.append((off, C - off))
    return out


def _build(C):
    """Build + compile the per-core program for capacity C (multiple of 128)."""
    assert C % P == 0
    nmc = C // P          # token m-tiles (for GEMM2 output rows)
    tiles = _tiles(C)

    nc = bacc.Bacc("TRN2", target_bir_lowering=False, debug=False,
                   num_devices=N_CORES)

    # DRAM I/O. Layouts are chosen so every DMA is >=2KB-contiguous per
    # partition line:
    #   xt[p, .]       = per-tile blocks [KK1, w] of X[c, kk*128+p] transposed
    #   w1[m, p, .]    = W1[kk*128+p, m*128+j] as [KK1, 128] per (m, p)
    #   w2[p, .]       = W2[kk*128+p, j] as [KK2, H]
    #   b1[p, m]       = b1[m*128+p]
    #   b2r[p, j]      = b2[j]            (pre-replicated across partitions)
    #   rw[p, mc]      = rweight[mc*128+p]
    #   y[mc, p, j]    = Y[mc*128+p, j]
    xt_d = nc.dram_tensor("xt", [P, KK1 * C], BF16, kind="ExternalInput")
    w1_d = nc.dram_tensor("w1", [MT1, P, KK1 * P], BF16, kind="ExternalInput")
    w2_d = nc.dram_tensor("w2", [P, KK2 * H], BF16, kind="ExternalInput")
    b1_d = nc.dram_tensor("b1", [P, MT1], F32, kind="ExternalInput")
    rw_d = nc.dram_tensor("rw", [P, nmc], F32, kind="ExternalInput")
    y_d = nc.dram_tensor("y", [nmc, P, H], BF16, kind="ExternalOutput")

    with tile.TileContext(nc) as tc:
        with (
            tc.tile_pool(name="const", bufs=1) as const,
            tc.tile_pool(name="xin", bufs=3) as xin,
            tc.tile_pool(name="gact", bufs=4) as gact,
            tc.tile_pool(name="yout", bufs=4) as yout,
            tc.tile_pool(name="psg", bufs=3, space="PSUM") as psg,
            tc.tile_pool(name="psy", bufs=5, space="PSUM") as psy,
        ):
            b1_sb = const.tile([P, MT1], F32)
            rw_sb = const.tile([P, nmc], F32)
            # One tile per weight chunk: Tile deps are per-tile, so a single
            # big w1 tile would make the first matmul wait for ALL 16 DMAs.
            w1m = [const.tile([P, KK1 * P], BF16, name=f"w1m{m}")
                   for m in range(MT1)]
            w2k = [const.tile([P, H], BF16, name=f"w2k{kk}")
                   for kk in range(KK2)]
            junk = const.tile([P, P], BF16)     # PE warm-up food
            junk2 = const.tile([P, NTILE], BF16)

            # Warm the PE's HAM clock gate while the first DMAs land, so the
            # real GEMM stream starts at 2.4 GHz with no idle window.
            nc.gpsimd.memset(junk[:], 0.0)
            nc.gpsimd.memset(junk2[:], 0.0)
            for i in range(20):
                pw = psy.tile([P, NTILE], F32, tag="py", name=f"pw{i}")
                nc.tensor.matmul(pw[:], junk[:], junk2[:],
                                 start=True, stop=True)

            xts = {}
            def xt_fetch(t, pieces=1, engs=None, separate=False):
                off, w = tiles[t]
                # pieces>1 fans the transfer across several DMA engines
                # (a single descriptor stream runs at ~145 GB/s);
                # separate=True makes each piece its own tile so matmuls
                # depend only on the kk-slices they actually read.
                if separate:
                    kkpp = KK1 // pieces
                    segs = []
                    for p_ in range(pieces):
                        seg = xin.tile([P, kkpp * w], BF16, tag=f"xtp{p_}",
                                       name=f"xt{t}_p{p_}")
                        nc.scalar.dma_start(
                            seg[:],
                            xt_d[:, KK1 * off + p_ * kkpp * w:
                                 KK1 * off + (p_ + 1) * kkpp * w])
                        segs.append(seg)
                    xts[t] = segs
                    return
                xts[t] = xin.tile([P, KK1 * w], BF16, tag="xt",
                                  name=f"xt{t}")
                span = KK1 * w
                step = span // pieces
                for p_ in range(pieces):
                    eng = (engs or [nc.scalar])[p_ % len(engs or [1])]
                    eng.dma_start(
                        xts[t][:, p_ * step:(p_ + 1) * step],
                        xt_d[:, KK1 * off + p_ * step:
                             KK1 * off + (p_ + 1) * step])

            # Two HWDGE queues: weights + even y-stores ride the Sync queue,
            # token tiles + odd y-stores ride the Activation queue.
            # Critical-path DMAs first: w1 m0/m1 + the first token tile,
            # then the weight refill stream (b1 after xt0 — it is not
            # needed until the first gelu eviction).
            nc.sync.dma_start(w1m[0][:], w1_d[0])
            nc.sync.dma_start(w1m[1][:], w1_d[1])
            xt_fetch(0, pieces=4)
            nc.scalar.dma_start(b1_sb[:], b1_d[:])
            for m in range(2, MT1):
                nc.sync.dma_start(w1m[m][:], w1_d[m])
            xt_fetch(1, pieces=2)
            nc.sync.dma_start(rw_sb[:], rw_d[:])
            for kk in range(KK2):
                nc.sync.dma_start(
                    w2k[kk][:], w2_d[:, kk * H:(kk + 1) * H])

            for t, (off, w) in enumerate(tiles):
                if t not in xts:
                    xt_fetch(t)
                if t + 2 < len(tiles) and t + 2 not in xts:
                    xt_fetch(t + 2)
                xt_sb = xts.pop(t)
                if isinstance(xt_sb, list):
                    kkpp = KK1 // len(xt_sb)
                    xseg = lambda kk: xt_sb[kk // kkpp][
                        :, (kk % kkpp) * w:(kk % kkpp + 1) * w]
                else:
                    xseg = lambda kk: xt_sb[:, kk * w:(kk + 1) * w]

                # GEMM1: GT[f, c] = sum_h W1[h, f] * XT[h, c], then
                # gelu(.+b1) on eviction. F on partitions, tokens on free.
                gt_sb = gact.tile([P, MT1 * w], BF16, tag="gt")
                for m in range(MT1):
                    pg = psg.tile([P, w], F32, tag="pg")
                    for kk in range(KK1):
                        nc.tensor.matmul(
                            pg[:],
                            w1m[m][:, kk * P:(kk + 1) * P],
                            xseg(kk),
                            start=(kk == 0), stop=(kk == KK1 - 1))
                    nc.scalar.activation(
                        gt_sb[:, m * w:(m + 1) * w], pg[:],
                        mybir.ActivationFunctionType.Gelu,
                        bias=b1_sb[:, m:m + 1])

                # GEMM2: Y[c, j] = sum_f GT[f, c] * W2[f, j]; tokens on
                # partitions. Evict with the routing-weight scale (b2 is
                # added host-side), then store the full H row-block with a
                # single DMA.
                for mo in range(w // P):
                    mc = off // P + mo
                    yo = yout.tile([P, H], BF16, tag="yo")
                    for n in range(NT2):
                        py = psy.tile([P, NTILE], F32, tag="py")
                        for kk in range(KK2):
                            nc.tensor.matmul(
                                py[:],
                                gt_sb[:, kk * w + mo * P:kk * w + (mo + 1) * P],
                                w2k[kk][:, n * NTILE:(n + 1) * NTILE],
                                start=(kk == 0), stop=(kk == KK2 - 1))
                        # evict on ACT: keeps the DVE out of the pipeline
                        # entirely (lower power; the eviction tick merges
                        # into the gelu tick stream the PE already tracks)
                        nc.scalar.activation(
                            yo[:, n * NTILE:(n + 1) * NTILE], py[:],
                            mybir.ActivationFunctionType.Copy,
                            scale=rw_sb[:, mc:mc + 1])
                    if mc >= nmc - 3:
                        # tail: split the store across both queues and four
                        # DMA engines so the final drain flushes fast
                        q = H // 4
                        for pi in range(4):
                            qe = nc.sync if pi % 2 == 0 else nc.scalar
                            qe.dma_start(y_d[mc, :, pi * q:(pi + 1) * q],
                                         yo[:, pi * q:(pi + 1) * q])
                    else:
                        qeng = nc.sync if mc % 2 == 0 else nc.scalar
                        qeng.dma_start(y_d[mc], yo[:])

    nc.compile()
    return nc


def kernel(hidden_states, expert_weights, top_experts, w1, b1, w2, b2,
           _trace=False):
    global last_result
    x = np.asarray(hidden_states, dtype=np.float32)
    fw = np.asarray(expert_weights, dtype=np.float32)
    te = np.asarray(top_experts).astype(np.int64)
    w1 = np.asarray(w1, dtype=np.float32)
    b1 = np.asarray(b1, dtype=np.float32)
    w2 = np.asarray(w2, dtype=np.float32)
    b2 = np.asarray(b2, dtype=np.float32)

    b, s, h = x.shape
    T = b * s
    xf = x.reshape(T, h)

    # Routed rows with same-expert dedup: row i<T is token i's slot-0 row
    # (combined weight when both slots agree); rows T.. are slot-1 rows of
    # the non-duplicate tokens.
    dup = te[:, 0] == te[:, 1]
    tok_nd = np.nonzero(~dup)[0]
    rows_tok = np.concatenate([np.arange(T), tok_nd])
    rows_e = np.concatenate([te[:, 0], te[tok_nd, 1]])
    rows_w = np.concatenate([np.where(dup, fw[:, 0] + fw[:, 1], fw[:, 0]),
                             fw[tok_nd, 1]])

    order = np.argsort(rows_e, kind="stable")
    counts = np.bincount(rows_e, minlength=E)
    starts = np.concatenate([[0], np.cumsum(counts)])
    C = max(int(-(-counts.max() // P)) * P, P)

    key = C
    if key not in _CACHE:
        _CACHE[key] = _build(C)
    nc = _CACHE[key]

    tiles = _tiles(C)
    nmc = C // P
    in_maps = []
    for e in range(E):
        idx = order[starts[e]:starts[e + 1]]
        cnt = len(idx)
        xe = np.zeros((C, H), np.float32)
        xe[:cnt] = xf[rows_tok[idx]]
        xt3 = np.ascontiguousarray(
            xe.T.reshape(KK1, P, C).transpose(1, 0, 2)).astype(
                ml_dtypes.bfloat16)                       # [P, KK1, C]
        xt = np.concatenate(
            [xt3[:, :, off:off + w].reshape(P, KK1 * w)
             for off, w in tiles], axis=1)                # [P, KK1*C]
        rwe = np.zeros(C, np.float32)
        rwe[:cnt] = rows_w[idx]
        w1e = np.ascontiguousarray(
            w1[e].reshape(KK1, P, MT1, P).transpose(2, 1, 0, 3).reshape(
                MT1, P, KK1 * P)).astype(ml_dtypes.bfloat16)
        in_maps.append({
            "xt": np.ascontiguousarray(xt),
            "w1": w1e,
            "w2": np.ascontiguousarray(
                w2[e].reshape(KK2, P, H).transpose(1, 0, 2).reshape(
                    P, KK2 * H)).astype(ml_dtypes.bfloat16),
            "b1": np.ascontiguousarray(b1[e].reshape(MT1, P).T),
            "rw": np.ascontiguousarray(rwe.reshape(nmc, P).T),
        })

    res = run_bass_kernel_spmd(nc, in_maps, list(range(N_CORES)),
                               trace=_trace)
    last_result = res

    nrows = len(rows_tok)
    routed = np.zeros((nrows, H), np.float32)
    for e in range(E):
        idx = order[starts[e]:starts[e + 1]]
        cnt = len(idx)
        ye = np.asarray(res.results[e]["y"]).astype(np.float32).reshape(
            C, H)[:cnt]
        # b2 is applied here (scaled by the routing weight) rather than on
        # the device, saving one vector op per eviction.
        routed[idx] = ye + np.outer(rows_w[idx], b2[e])

    y = routed[:T]
    y[tok_nd] += routed[T:]
    return y.reshape(b, s, h).astype(np.float32)
